# revision 6
# baseline (speedup 1.0000x reference)
"""Distributed GQA attention kernel for 8 TRN2 NeuronCores.

Problem: B=1, S=2048, D=4096, H=32 q-heads, KV=8 kv-heads, HD=128.
  q = rope(x@wq.T), k = rope(x@wk.T), v = x@wv.T
  out = softmax(causal(q@k.T/sqrt(HD))) @ v @ wo.T

Sharding: tensor-parallel over heads. Core c owns q-heads 4c..4c+3 and
kv-head c. Device-side per core:
  phase 1: QT/KT (rope'd, [hd, s] layout) + V ([t, hd]) projections
  phase 2: causal attention producing attnT [512, 2048] (bf16)
  phase 3: AllGather attnT -> [4096, 2048]; out.T slice = wot.T @ attnT
Host side: layout prep (transposes, bf16 cast, rope tables) + final
concat/transpose of the 8 out.T slices.
"""

import math
import numpy as np
import ml_dtypes

BF = ml_dtypes.bfloat16

B, S, D = 1, 2048, 4096
H, KV, HD = 32, 8, 128
NCORES = 8
HL = H // NCORES            # 4 local q heads
QW = HL * HD                # 512 local q width
SC = 512                    # s-chunk width
NSC = S // SC               # 4 s-chunks
KD = 32                     # d-dim k-tiles (4096/128)
NT = S // 128               # 16 t-tiles
SCALE = 1.0 / math.sqrt(HD)
NEG = -30000.0


def _build_nc():
    import concourse.bass as bass
    import concourse.mybir as mybir
    from concourse import bacc, tile

    dt = mybir.dt
    nc = bacc.Bacc()

    xt_d = nc.declare_dram_parameter("xt", [D, S], dt.bfloat16, isOutput=False)
    wqt_d = nc.declare_dram_parameter("wqt", [D, QW], dt.bfloat16, isOutput=False)
    wkt_d = nc.declare_dram_parameter("wkt", [D, HD], dt.bfloat16, isOutput=False)
    wvt_d = nc.declare_dram_parameter("wvt", [D, HD], dt.bfloat16, isOutput=False)
    wot_d = nc.declare_dram_parameter("wot", [D, QW], dt.bfloat16, isOutput=False)
    cosd_d = nc.declare_dram_parameter("cosd", [HD, S], dt.bfloat16, isOutput=False)
    sind_d = nc.declare_dram_parameter("sind", [HD, S], dt.bfloat16, isOutput=False)
    swapt_d = nc.declare_dram_parameter("swapt", [HD, HD], dt.bfloat16, isOutput=False)
    ident_d = nc.declare_dram_parameter("ident", [HD, HD], dt.bfloat16, isOutput=False)
    dmask_d = nc.declare_dram_parameter("dmask", [128, 128], dt.float32, isOutput=False)
    onesc_d = nc.declare_dram_parameter("onesc", [128, 1], dt.bfloat16, isOutput=False)
    onesr_d = nc.declare_dram_parameter("onesr", [1, 128], dt.float32, isOutput=False)
    out_d = nc.declare_dram_parameter("out_t", [QW, S], dt.float32, isOutput=True)

    with tile.TileContext(nc) as tc:
        with (
            tc.tile_pool(name="const", bufs=1) as cpool,
            tc.tile_pool(name="qkv", bufs=1) as qkvpool,
            tc.tile_pool(name="att", bufs=1) as attpool,
            tc.tile_pool(name="dram", bufs=1, space="DRAM") as dpool,
        ):
            # ---- resident constants / weights ----
            wqt = cpool.tile([128, KD, QW], dt.bfloat16)
            wkt = cpool.tile([128, KD, HD], dt.bfloat16)
            wvt = cpool.tile([128, KD, HD], dt.bfloat16)
            cosd = cpool.tile([HD, S], dt.bfloat16)
            sind = cpool.tile([HD, S], dt.bfloat16)
            swapt = cpool.tile([HD, HD], dt.bfloat16)
            ident = cpool.tile([HD, HD], dt.bfloat16)
            dmask = cpool.tile([128, 128], dt.float32)
            onesc = cpool.tile([128, 1], dt.bfloat16)
            onesr = cpool.tile([1, 128], dt.float32)

            nc.sync.dma_start(wqt[:], wqt_d[:, :].rearrange("(k p) n -> p k n", p=128))
            nc.sync.dma_start(wkt[:], wkt_d[:, :].rearrange("(k p) n -> p k n", p=128))
            nc.sync.dma_start(wvt[:], wvt_d[:, :].rearrange("(k p) n -> p k n", p=128))
            nc.sync.dma_start(cosd[:], cosd_d[:, :])
            nc.sync.dma_start(sind[:], sind_d[:, :])
            nc.sync.dma_start(swapt[:], swapt_d[:, :])
            nc.sync.dma_start(ident[:], ident_d[:, :])
            nc.sync.dma_start(dmask[:], dmask_d[:, :])
            nc.sync.dma_start(onesc[:], onesc_d[:, :])
            nc.sync.dma_start(onesr[:], onesr_d[:, :])

            # ---- persistent activations ----
            qt = [qkvpool.tile([HD, S], dt.bfloat16, name=f"qt{h}", tag=f"qt{h}")
                  for h in range(HL)]
            kt = qkvpool.tile([HD, S], dt.bfloat16)
            vv = qkvpool.tile([128, NT, HD], dt.bfloat16)   # [t_part, ti, hd]
            att = [attpool.tile([HD, S], dt.bfloat16, name=f"att{h}", tag=f"att{h}")
                   for h in range(HL)]

            xt_r = xt_d[:, :].rearrange("(k p) s -> p k s", p=128)

            # ================= phase 1: projections + rope =================
            with (
                tc.tile_pool(name="xc", bufs=2) as xpool,
                tc.tile_pool(name="p1", bufs=3, space="PSUM") as pp1,
                tc.tile_pool(name="pr", bufs=2, space="PSUM") as ppr,
                tc.tile_pool(name="rtmp", bufs=3) as rtpool,
            ):
                for sc in range(NSC):
                    ssl = slice(sc * SC, (sc + 1) * SC)
                    xc = xpool.tile([128, KD, SC], dt.bfloat16)
                    nc.sync.dma_start(xc[:], xt_r[:, :, ssl])

                    # Q heads and K: produce rope'd [hd, s] rows
                    for hi in range(HL + 1):
                        ps = pp1.tile([128, SC], dt.float32)
                        for k in range(KD):
                            if hi < HL:
                                lhs = wqt[:, k, hi * HD:(hi + 1) * HD]
                            else:
                                lhs = wkt[:, k, :]
                            nc.tensor.matmul(ps[:], lhs, xc[:, k, :],
                                             start=(k == 0), stop=(k == KD - 1))
                        # rope: out = q*cos + rot(q)*sin, rot via swap-matmul
                        qs = rtpool.tile([128, SC], dt.bfloat16, tag="ropeqs")
                        qc = rtpool.tile([128, SC], dt.bfloat16, tag="ropeqc")
                        nc.vector.tensor_mul(qs[:], ps[:], sind[:, ssl])
                        nc.vector.tensor_mul(qc[:], ps[:], cosd[:, ssl])
                        ps2 = ppr.tile([128, SC], dt.float32)
                        nc.tensor.matmul(ps2[:], swapt[:], qs[:], start=True, stop=False)
                        nc.tensor.matmul(ps2[:], ident[:], qc[:], start=False, stop=True)
                        dst = qt[hi] if hi < HL else kt
                        nc.scalar.copy(dst[:, ssl], ps2[:])

                    # V tiles in [t, hd] layout
                    for vt in range(4):
                        ti = sc * 4 + vt
                        psv = pp1.tile([128, SC], dt.float32, tag="vps")
                        for k in range(KD):
                            nc.tensor.matmul(psv[:, :HD],
                                             xc[:, k, vt * 128:(vt + 1) * 128],
                                             wvt[:, k, :],
                                             start=(k == 0), stop=(k == KD - 1))
                        nc.scalar.copy(vv[:, ti, :], psv[:, :HD])

            # ================= phase 2: causal attention =================
            with (
                tc.tile_pool(name="st", bufs=3, space="PSUM") as stpool,
                tc.tile_pool(name="pv", bufs=2, space="PSUM") as pvpool,
                tc.tile_pool(name="rs", bufs=2, space="PSUM") as rspool,
                tc.tile_pool(name="bc", bufs=1, space="PSUM") as bcpool,
                tc.tile_pool(name="pt", bufs=4) as ptpool,
                tc.tile_pool(name="ep", bufs=2) as eppool,
            ):
                for sc in range(NSC):
                    ssl = slice(sc * SC, (sc + 1) * SC)
                    n_t = sc * 4 + 4
                    for h in range(HL):
                        pv = pvpool.tile([128, SC], dt.float32)
                        rs = rspool.tile([1, SC], dt.float32)
                        for ti in range(n_t):
                            st = stpool.tile([128, SC], dt.float32)
                            nc.tensor.matmul(st[:], kt[:, ti * 128:(ti + 1) * 128],
                                             qt[h][:, ssl], start=True, stop=True)
                            d_off = ti * 128 - sc * SC
                            if d_off >= 0:
                                nc.vector.tensor_add(st[:, d_off:d_off + 128],
                                                     st[:, d_off:d_off + 128],
                                                     dmask[:])
                            pt = ptpool.tile([128, SC], dt.bfloat16)
                            nc.scalar.activation(pt[:], st[:],
                                                 mybir.ActivationFunctionType.Exp,
                                                 scale=SCALE)
                            if d_off > 0:
                                nc.gpsimd.memset(pt[:, :d_off], 0.0)
                            nc.tensor.matmul(rs[:], onesc[:], pt[:],
                                             start=(ti == 0), stop=(ti == n_t - 1))
                            nc.tensor.matmul(pv[:], vv[:, ti, :], pt[:],
                                             start=(ti == 0), stop=(ti == n_t - 1))
                        # epilogue: normalize columns by 1/rowsum
                        rec = eppool.tile([1, SC], dt.float32, tag="rec")
                        nc.vector.reciprocal(rec[:], rs[:])
                        bc = bcpool.tile([128, SC], dt.float32)
                        nc.tensor.matmul(bc[:], onesr[:], rec[:], start=True, stop=True)
                        bcs = eppool.tile([128, SC], dt.float32, tag="bcs")
                        nc.scalar.copy(bcs[:], bc[:])
                        nc.vector.tensor_mul(att[h][:, ssl], pv[:], bcs[:])

            # ================= phase 3: allgather + out-proj =================
            ag_in = dpool.tile([QW, S], dt.bfloat16)
            ag_out = dpool.tile([NCORES * QW, S], dt.bfloat16,
                                addr_space="Shared")
            for h in range(HL):
                nc.sync.dma_start(ag_in[h * HD:(h + 1) * HD, :], att[h][:])
            nc.gpsimd.collective_compute(
                "AllGather",
                mybir.AluOpType.bypass,
                replica_groups=[list(range(NCORES))],
                ins=[ag_in.opt()],
                outs=[ag_out.opt()],
            )
            ag_r = ag_out[:, :].rearrange("(k p) s -> p k s", p=128)

            with (
                tc.tile_pool(name="wo", bufs=1) as wopool,
                tc.tile_pool(name="agc", bufs=2) as agpool,
                tc.tile_pool(name="p3", bufs=3, space="PSUM") as pp3,
                tc.tile_pool(name="o3", bufs=3) as opool,
            ):
                wot = wopool.tile([128, KD, QW], dt.bfloat16)
                nc.sync.dma_start(wot[:], wot_d[:, :].rearrange("(k p) n -> p k n", p=128))
                for sc in range(NSC):
                    ssl = slice(sc * SC, (sc + 1) * SC)
                    agc = agpool.tile([128, KD, SC], dt.bfloat16)
                    nc.sync.dma_start(agc[:], ag_r[:, :, ssl])
                    for oc in range(4):
                        ps = pp3.tile([128, SC], dt.float32)
                        for k in range(KD):
                            nc.tensor.matmul(ps[:], wot[:, k, oc * 128:(oc + 1) * 128],
                                             agc[:, k, :],
                                             start=(k == 0), stop=(k == KD - 1))
                        ot = opool.tile([128, SC], dt.float32)
                        nc.vector.tensor_copy(ot[:], ps[:])
                        nc.sync.dma_start(out_d[oc * 128:(oc + 1) * 128, ssl], ot[:])
    if not nc.is_finalized():
        nc.finalize()
    return nc


_CACHE = {}


def _get_nc():
    if "nc" not in _CACHE:
        _CACHE["nc"] = _build_nc()
    return _CACHE["nc"]


def _prep_in_maps(x, wq, wk, wv, wo, freqs_cos, freqs_sin):
    xt = np.ascontiguousarray(x.reshape(S, D).T).astype(BF)
    cosd = np.repeat(np.asarray(freqs_cos, np.float32).T, 2, axis=0).astype(BF)
    sind = np.repeat(np.asarray(freqs_sin, np.float32).T, 2, axis=0).astype(BF)
    swapt = np.zeros((HD, HD), np.float32)
    for i in range(HD // 2):
        swapt[2 * i + 1, 2 * i] = -1.0
        swapt[2 * i, 2 * i + 1] = 1.0
    swapt = swapt.astype(BF)
    ident = np.eye(HD, dtype=np.float32).astype(BF)
    t_idx = np.arange(128)[:, None]
    s_idx = np.arange(128)[None, :]
    dmask = np.where(s_idx >= t_idx, 0.0, NEG).astype(np.float32)
    onesc = np.ones((128, 1), np.float32).astype(BF)
    onesr = np.ones((1, 128), np.float32)

    wq = np.asarray(wq, np.float32)
    wk = np.asarray(wk, np.float32)
    wv = np.asarray(wv, np.float32)
    wo = np.asarray(wo, np.float32)

    in_maps = []
    for c in range(NCORES):
        qsl = slice(QW * c, QW * (c + 1))
        ksl = slice(HD * c, HD * (c + 1))
        in_maps.append({
            "xt": xt,
            "wqt": np.ascontiguousarray(wq[qsl].T).astype(BF),
            "wkt": np.ascontiguousarray(wk[ksl].T).astype(BF),
            "wvt": np.ascontiguousarray(wv[ksl].T).astype(BF),
            "wot": np.ascontiguousarray(wo[qsl].T).astype(BF),
            "cosd": cosd, "sind": sind, "swapt": swapt, "ident": ident,
            "dmask": dmask, "onesc": onesc, "onesr": onesr,
        })
    return in_maps


def run(inputs, trace=False):
    from concourse.bass_utils import run_bass_kernel_spmd
    nc = _get_nc()
    in_maps = _prep_in_maps(
        inputs["x"], inputs["wq"], inputs["wk"], inputs["wv"], inputs["wo"],
        inputs["freqs_cos"], inputs["freqs_sin"])
    res = run_bass_kernel_spmd(nc, in_maps, core_ids=list(range(NCORES)),
                               trace=trace)
    shards = [np.asarray(res.results[c]["out_t"], np.float32)
              for c in range(NCORES)]
    full = np.concatenate(shards, axis=0)          # [4096, 2048]
    out = np.ascontiguousarray(full.T)[None]       # [1, 2048, 4096]
    return out.astype(np.float32), res


def kernel(**inputs):
    out, _ = run(inputs, trace=False)
    return out
